# revision 1
# baseline (speedup 1.0000x reference)
"""Trainium2 Bass kernel for nn_CustomLoss_74826920231413.

Loss structure (B=32, E=1024, K=20):
    c  = complex(nnOutput[:, :NOUT], nnOutput[:, NOUT:])
    d  = c[:, :K];  U = c[:, K:VLOC].reshape(B,E,K);  V = c[:, VLOC:].reshape(B,E,K)
    obj1/obj2 = sum_{j<k} |U^T U| / B (no conj), same for V
    pred = U @ diag(d) @ V^T;  tk = complex(kern_real, kern_imag)
    loss = ||tk - pred||^2 / ||tk||^2 + 0.01*(obj1+obj2)

Device strategy (data-parallel over B, 4 batch rows per core, 8 cores):
    ||tk - pred||^2 = ||tk||^2 - 2*Re<conj(tk),pred> + ||pred||^2, so the
    device only needs one streaming pass over tk producing small outputs:
      * gram[b]  = [Ur|Ui]^T[Ur|Ui] and [Vr|Vi]^T[Vr|Vi]  -> objs, ||pred||^2
      * yr[b]    = W^T tkr with W = [Ur|Ui]      (40x1024) -> cross term
      * yi[b]    = W^T tki                        (40x1024)
      * den partials = per-partition sums of tk^2
    Host assembles the three scalars from these partials in float64.

    tk is shipped to the device as fp16: the loss is a ratio of O(1e9)
    quantities and 16-bit rounding of tk perturbs it at ~1e-6 relative
    (validated numerically), while halving the dominant DMA traffic.
    Gram runs in exact fp32 from the fp32 nnOutput. All input streams ride
    the sync HWDGE ring in host-prepacked partition-major layout (16KB
    contiguous lines); output stores ride gpsimd SWDGE queues.
"""

import sys

for _p in ("/opt/trn_rl_repo", "/root/.axon_site/_ro/trn_rl_repo"):
    if _p not in sys.path:
        sys.path.append(_p)

import numpy as np

import concourse.bacc as bacc
import concourse.mybir as mybir
import concourse.tile as tile
from concourse.bass_utils import run_bass_kernel_spmd

# Problem constants (hardcoded per harness contract)
E = 1024
K = 20
NOUT = K * (2 * E + 1)          # 40980
VLOC = K + K * E                # 20500
PENALTY = 0.01
B = 32
NCORES = 8
NB = B // NCORES                # batch rows per core
NCH = E // 128                  # 8 e-chunks of 128 partitions
HALF = NCH // 2                 # tk DMA split granularity (chunks per DMA)
F32 = mybir.dt.float32
F16 = mybir.dt.float16

_PROGRAM_CACHE = {}


def _build_program():
    """Per-core SPMD Bass program. Same program on all 8 cores; each core
    receives its own 4-row slice of the inputs (host-packed layouts)."""
    nc = bacc.Bacc("TRN2", target_bir_lowering=False, debug=False)

    # host-packed [Ur|Ui|Vr|Vi] fp32, partition-major: [b, p, c, 80]
    xuv_d = nc.dram_tensor("xuv", [NB, 128, NCH, 80], F32, kind="ExternalInput").ap()
    # host-packed fp16 [Ur|Ui] weights: [b, p, c, 40]
    w_d = nc.dram_tensor("w16", [NB, 128, NCH, 40], F16, kind="ExternalInput").ap()
    # host-packed fp16 kernels, partition-major: [b, p, c, f], e = c*128+p.
    # 16KB contiguous per partition line -> few DMA descriptors, so a single
    # HWDGE ring feeds the full HBM bandwidth.
    tkr_d = nc.dram_tensor("tkr", [NB, 128, NCH, E], F16, kind="ExternalInput").ap()
    tki_d = nc.dram_tensor("tki", [NB, 128, NCH, E], F16, kind="ExternalInput").ap()

    gram_d = nc.dram_tensor("gram", [NB, 40, 80], F32, kind="ExternalOutput").ap()
    yr_d = nc.dram_tensor("yr", [NB, 40, E], F32, kind="ExternalOutput").ap()
    yi_d = nc.dram_tensor("yi", [NB, 40, E], F32, kind="ExternalOutput").ap()
    den_d = nc.dram_tensor(
        "den", [2, 128, NB * NCH * 2], F32, kind="ExternalOutput"
    ).ap()

    mult = mybir.AluOpType.mult
    Square = mybir.ActivationFunctionType.Square

    with tile.TileContext(nc) as tc:
        with (
            tc.tile_pool(name="xuv", bufs=2) as xpool,
            tc.tile_pool(name="tk", bufs=3) as tkpool,
            tc.tile_pool(name="scr", bufs=2) as scrpool,
            tc.tile_pool(name="evac", bufs=2) as evacpool,
            tc.tile_pool(name="den", bufs=1) as denpool,
            tc.tile_pool(name="psg", bufs=2, space="PSUM") as psg_pool,
            tc.tile_pool(name="psy", bufs=1, space="PSUM") as psy_pool,
        ):
            # den accumulator columns; each engine owns its own tile (no
            # cross-engine write conflicts). col = (b*NCH + c)*2 + mat
            den_dve = denpool.tile([128, NB * NCH * 2], F32, name="den_dve")
            den_act = denpool.tile([128, NB * NCH * 2], F32, name="den_act")
            nc.vector.memset(den_dve[:], 0.0)
            nc.vector.memset(den_act[:], 0.0)

            for b in range(NB):
                # ---- kernels, fp16, halves for pipelining: [p, c, f]
                tkr_sb = []
                tki_sb = []
                for h in range(NCH // HALF):
                    cs = slice(h * HALF, (h + 1) * HALF)
                    tr = tkpool.tile([128, HALF, E], F16, name=f"tkr_h{h}")
                    nc.sync.dma_start(tr[:], tkr_d[b, :, cs])
                    tkr_sb.append(tr)
                    ti = tkpool.tile([128, HALF, E], F16, name=f"tki_h{h}")
                    nc.sync.dma_start(ti[:], tki_d[b, :, cs])
                    tki_sb.append(ti)

                def tkr_c(c):
                    return tkr_sb[c // HALF][:, c % HALF, :]

                def tki_c(c):
                    return tki_sb[c // HALF][:, c % HALF, :]

                # ---- U/V tile (fp32) + fp16 Y weights, host-packed layouts
                x_sb = xpool.tile([128, NCH, 80], F32, name="x_sb")
                nc.sync.dma_start(x_sb[:], xuv_d[b])
                w_sb = xpool.tile([128, NCH, 40], F16, name="w_sb")
                nc.sync.dma_start(w_sb[:], w_d[b])

                # ---- Grams: S_U = [Ur|Ui]^T [Ur|Ui], S_V likewise (exact
                # fp32). The U-V cross blocks are never needed by the host.
                ps_g = psg_pool.tile([40, 80], F32, name="ps_g")
                for c in range(NCH):
                    xu = x_sb[:, c, 0:40]
                    nc.tensor.matmul(
                        ps_g[:, 0:40], xu, xu, start=(c == 0), stop=(c == NCH - 1)
                    )
                for c in range(NCH):
                    xv = x_sb[:, c, 40:80]
                    nc.tensor.matmul(
                        ps_g[:, 40:80], xv, xv, start=(c == 0), stop=(c == NCH - 1)
                    )
                g_sb = evacpool.tile([40, 80], F32, name="g_sb")
                nc.vector.tensor_copy(g_sb[:], ps_g[:])
                nc.gpsimd.dma_start(gram_d[b], g_sb[:])

                # ---- Y: yr[j,f] = sum_e W[e,j] tkr[e,f], W = [Ur|Ui] (fp16)
                ps_yr = psy_pool.tile([40, E], F32, name="ps_yr")
                ps_yi = psy_pool.tile([40, E], F32, name="ps_yi")
                for c in range(NCH):
                    w = w_sb[:, c, :]
                    for h in range(2):
                        fs = slice(h * 512, (h + 1) * 512)
                        nc.tensor.matmul(
                            ps_yr[:, fs],
                            w,
                            tkr_c(c)[:, fs],
                            start=(c == 0),
                            stop=(c == NCH - 1),
                        )
                        nc.tensor.matmul(
                            ps_yi[:, fs],
                            w,
                            tki_c(c)[:, fs],
                            start=(c == 0),
                            stop=(c == NCH - 1),
                        )
                yr_sb = evacpool.tile([40, E], F32, name="yr_sb")
                nc.scalar.copy(yr_sb[:], ps_yr[:])
                nc.gpsimd.dma_start(yr_d[b], yr_sb[:])
                yi_sb = evacpool.tile([40, E], F32, name="yi_sb")
                nc.scalar.copy(yi_sb[:], ps_yi[:])
                nc.gpsimd.dma_start(yi_d[b], yi_sb[:])

                # ---- den partials: sum of squares along free dim (fp32
                # accumulate). Alternate units between DVE and ACT so the
                # post-stream straggler work is split across both engines.
                for c in range(NCH):
                    for mat, src in ((0, tkr_c(c)), (1, tki_c(c))):
                        idx = c * 2 + mat
                        col = (b * NCH + c) * 2 + mat
                        if idx % 2 == 0:
                            scr_v = scrpool.tile([128, E], F16, name="scr_v")
                            nc.vector.scalar_tensor_tensor(
                                scr_v[:],
                                src,
                                1.0,
                                src,
                                mult,
                                mult,
                                accum_out=den_dve[:, col:col + 1],
                            )
                        else:
                            scr_a = scrpool.tile([128, E], F16, name="scr_a")
                            nc.scalar.activation(
                                scr_a[:],
                                src,
                                Square,
                                accum_out=den_act[:, col:col + 1],
                            )

            nc.gpsimd.dma_start(den_d[0], den_dve[:])
            nc.gpsimd.dma_start(den_d[1], den_act[:])

    nc.compile()
    return nc


def _get_program():
    if "nc" not in _PROGRAM_CACHE:
        _PROGRAM_CACHE["nc"] = _build_program()
    return _PROGRAM_CACHE["nc"]


def _pack_inputs(nn, tkr, tki):
    """Host-side packing: per-core input dicts with device-friendly layouts."""
    # partition-major fp16: [B, E, E] -> [B, p, c, f] with e = c*128 + p
    tkr16 = np.ascontiguousarray(
        tkr.astype(np.float16).reshape(B, NCH, 128, E).transpose(0, 2, 1, 3)
    )
    tki16 = np.ascontiguousarray(
        tki.astype(np.float16).reshape(B, NCH, 128, E).transpose(0, 2, 1, 3)
    )
    # [B, E, K] slices of nn
    Ur = nn[:, K:VLOC].reshape(B, E, K)
    Ui = nn[:, NOUT + K:NOUT + VLOC].reshape(B, E, K)
    Vr = nn[:, VLOC:NOUT].reshape(B, E, K)
    Vi = nn[:, NOUT + VLOC:2 * NOUT].reshape(B, E, K)
    xuv = np.concatenate([Ur, Ui, Vr, Vi], axis=2)        # [B, E, 80] f32
    # partition-major: e = c*128 + p  ->  [B, p, c, 80]
    xuv = np.ascontiguousarray(
        xuv.reshape(B, NCH, 128, 80).transpose(0, 2, 1, 3)
    )
    w16 = np.ascontiguousarray(
        np.concatenate([Ur, Ui], axis=2)
        .reshape(B, NCH, 128, 40)
        .transpose(0, 2, 1, 3)
        .astype(np.float16)
    )
    return [
        {
            "xuv": xuv[i * NB:(i + 1) * NB],
            "w16": w16[i * NB:(i + 1) * NB],
            "tkr": tkr16[i * NB:(i + 1) * NB],
            "tki": tki16[i * NB:(i + 1) * NB],
        }
        for i in range(NCORES)
    ]


def _run_device(nn, tkr, tki, trace=False):
    nc = _get_program()
    in_maps = _pack_inputs(nn, tkr, tki)
    return run_bass_kernel_spmd(nc, in_maps, list(range(NCORES)), trace=trace)


def _finalize(nn, results, batch_size):
    """Assemble (loss, obj1, obj2) from per-core device partials (float64)."""
    nn = np.asarray(nn)
    d = (nn[:, :K] + 1j * nn[:, NOUT:NOUT + K]).astype(np.complex128)
    Vr = nn[:, VLOC:NOUT].reshape(B, E, K).astype(np.float64)
    Vi = nn[:, NOUT + VLOC:2 * NOUT].reshape(B, E, K).astype(np.float64)
    V = Vr + 1j * Vi

    gram = np.concatenate(
        [r["gram"] for r in results], axis=0
    ).astype(np.float64)                                   # [B, 40, 80]
    yr = np.concatenate([r["yr"] for r in results], axis=0).astype(np.float64)
    yi = np.concatenate([r["yi"] for r in results], axis=0).astype(np.float64)
    den = float(sum(np.sum(r["den"], dtype=np.float64) for r in results))

    SU = gram[:, :, 0:40]
    SV = gram[:, :, 40:80]
    Srr = SU[:, 0:20, 0:20]
    Sri = SU[:, 0:20, 20:40]
    Sii = SU[:, 20:40, 20:40]
    Trr = SV[:, 0:20, 0:20]
    Tri = SV[:, 0:20, 20:40]
    Tii = SV[:, 20:40, 20:40]
    SriT = np.transpose(Sri, (0, 2, 1))
    TriT = np.transpose(Tri, (0, 2, 1))
    G_U = (Srr - Sii) + 1j * (Sri + SriT)
    G_V = (Trr - Tii) + 1j * (Tri + TriT)
    H_U = (Srr + Sii) + 1j * (Sri - SriT)
    H_V = (Trr + Tii) + 1j * (Tri - TriT)

    mask = np.triu(np.ones((K, K), dtype=bool), k=1)
    bsz = float(batch_size)
    obj1 = float(np.sum(np.abs(G_U)[:, mask]) / bsz)
    obj2 = float(np.sum(np.abs(G_V)[:, mask]) / bsz)

    prednorm = float(
        np.real(
            np.einsum("bk,bl,bkl,bkl->", d, np.conj(d), np.conj(H_U), np.conj(H_V))
        )
    )

    # cross = Re<conj(tk), pred>; Wc[b,k,f] = sum_e conj(tk[e,f]) U[e,k]
    Wc = (yr[:, 0:20, :] + yi[:, 20:40, :]) + 1j * (yr[:, 20:40, :] - yi[:, 0:20, :])
    zeta = np.einsum("bfk,bkf->bk", V, Wc)
    cross = float(np.real(np.einsum("bk,bk->", d, zeta)))

    num = den - 2.0 * cross + prednorm
    loss = num / den + PENALTY * (obj1 + obj2)
    return (
        np.float32(loss),
        np.float32(obj1),
        np.float32(obj2),
    )


def kernel(nnOutput, kern_real, kern_imag, batch_Size):
    nn = np.ascontiguousarray(np.asarray(nnOutput, dtype=np.float32))
    tkr = np.asarray(kern_real, dtype=np.float32)
    tki = np.asarray(kern_imag, dtype=np.float32)
    res = _run_device(nn, tkr, tki).results
    return _finalize(nn, res, int(batch_Size))



# revision 2
# speedup vs baseline: 1.5643x; 1.5643x over previous
"""Trainium2 Bass kernel for nn_CustomLoss_74826920231413.

Loss structure (B=32, E=1024, K=20):
    c  = complex(nnOutput[:, :NOUT], nnOutput[:, NOUT:])
    d  = c[:, :K];  U = c[:, K:VLOC].reshape(B,E,K);  V = c[:, VLOC:].reshape(B,E,K)
    obj1/obj2 = sum_{j<k} |U^T U| / B (no conj), same for V
    pred = U @ diag(d) @ V^T;  tk = complex(kern_real, kern_imag)
    loss = ||tk - pred||^2 / ||tk||^2 + 0.01*(obj1+obj2)

Device strategy (data-parallel over B, 4 batch rows per core, 8 cores):
    ||tk - pred||^2 = ||tk||^2 - 2*Re<conj(tk),pred> + ||pred||^2.  The
    device streams tk once, producing small outputs:
      * gram[b] = X^T X with X = [Ur|Ui|Vr|Vi]   -> objs, ||pred||^2
      * yr[b]   = W^T tkr with W = [Ur|Ui]       -> cross term
      * yi[b]   = W^T tki
    den = ||tk||^2 is an exact scalar the host computes in float64 from
    the fp32 originals (cheap BLAS dot); the loss is a ratio of O(1e9)
    sums, so the cross/pred partials tolerate aggressive quantization:
      * tk rides as fp8 e4m3 (validated: perturbs the loss at ~1e-6
        relative) -> halves the dominant HBM traffic vs fp16.
      * Y matmuls run in fp8 DoubleRow perf mode (2 k-chunks per
        instruction) to keep PE comfortably under the DMA envelope.
      * xuv (U/V for the Gram) rides fp16; Gram runs on fp16 inputs
        (PE computes in fp22/fp32) which keeps obj1/obj2 at ~1e-5 rel.
    tkr streams on the sync HWDGE ring, tki on the act HWDGE ring, so
    both hardware descriptor rings feed HBM concurrently.  Everything
    is SBUF-resident (~90KB/partition), so all input DMAs are issued
    up-front and the rings run back-to-back at full rate with the PE
    trailing behind; outputs ride gpsimd SWDGE queues.
"""

import sys

for _p in ("/opt/trn_rl_repo", "/root/.axon_site/_ro/trn_rl_repo"):
    if _p not in sys.path:
        sys.path.append(_p)

import ml_dtypes
import numpy as np

import concourse.bacc as bacc
import concourse.mybir as mybir
import concourse.tile as tile
from concourse.bass_utils import run_bass_kernel_spmd

# Problem constants (hardcoded per harness contract)
E = 1024
K = 20
NOUT = K * (2 * E + 1)          # 40980
VLOC = K + K * E                # 20500
PENALTY = 0.01
B = 32
NCORES = 8
NB = B // NCORES                # batch rows per core
NCH = E // 128                  # 8 e-chunks of 128 partitions
NJ = NCH // 2                   # 4 chunk-pairs (DoubleRow contracts 2)
F32 = mybir.dt.float32
F16 = mybir.dt.float16
BF16 = mybir.dt.bfloat16
FP8 = mybir.dt.float8e4
FP8_NP = ml_dtypes.float8_e4m3fn
DR = mybir.MatmulPerfMode.DoubleRow

_PROGRAM_CACHE = {}


def _build_program():
    """Per-core SPMD Bass program. Same program on all 8 cores; each core
    receives its own 4-row slice of the inputs (host-packed layouts)."""
    nc = bacc.Bacc("TRN2", target_bir_lowering=False, debug=False)

    # host-packed [Ur|Ui|Vr|Vi] fp16, partition-major: [b, p, c, 80]
    xuv_d = nc.dram_tensor("xuv", [NB, 128, NCH, 80], F16, kind="ExternalInput").ap()
    # host-packed fp8 kernels: [b, p, h, c, f512], e = c*128+p, f = h*512+f512.
    # 4KB contiguous per (b,p,h) line -> efficient DMA descriptors.
    tkr_d = nc.dram_tensor(
        "tkr", [NB, 128, 2, NCH, 512], FP8, kind="ExternalInput"
    ).ap()
    tki_d = nc.dram_tensor(
        "tki", [NB, 128, 2, NCH, 512], FP8, kind="ExternalInput"
    ).ap()

    gram_d = nc.dram_tensor("gram", [NB, 80, 80], F32, kind="ExternalOutput").ap()
    yr_d = nc.dram_tensor("yr", [NB, 40, E], BF16, kind="ExternalOutput").ap()
    yi_d = nc.dram_tensor("yi", [NB, 40, E], BF16, kind="ExternalOutput").ap()

    with tile.TileContext(nc) as tc:
        with (
            tc.tile_pool(name="x", bufs=1) as xpool,
            tc.tile_pool(name="tk", bufs=1) as tkpool,
            tc.tile_pool(name="ev", bufs=1) as evpool,
            tc.tile_pool(name="psg", bufs=2, space="PSUM") as psgp,
            tc.tile_pool(name="psy", bufs=6, space="PSUM") as psyp,
        ):
            # ---- all input DMAs issued up-front, in consumption order.
            # sync ring: xuv then tkr halves; act ring: tki halves.
            x_sb = []
            for b in range(NB):
                t = xpool.tile([128, NCH, 80], F16, name=f"x{b}")
                nc.sync.dma_start(t[:], xuv_d[b])
                x_sb.append(t)
            tk_sb = {}
            for b in range(NB):
                for h in range(2):
                    t = tkpool.tile([128, NCH, 512], FP8, name=f"tkr{b}_{h}")
                    nc.sync.dma_start(t[:], tkr_d[b, :, h])
                    tk_sb[(b, 0, h)] = t
                    t = tkpool.tile([128, NCH, 512], FP8, name=f"tki{b}_{h}")
                    nc.scalar.dma_start(t[:], tki_d[b, :, h])
                    tk_sb[(b, 1, h)] = t

            # ---- fp8 Y-weights derived on device from xuv (cols 0:40 =
            # [Ur|Ui]); pad cols 40:48 are never read back (psum rows
            # 40:48 are junk and only rows 0:40 get evacuated).
            w8 = []
            for b in range(NB):
                t = xpool.tile([128, NCH, 48], FP8, name=f"w{b}")
                nc.vector.tensor_copy(t[:, :, 0:40], x_sb[b][:, :, 0:40])
                w8.append(t)

            # ---- Grams: [80,80] = X^T X per b (fp16 in, fp32 accum).
            for b in range(NB):
                ps_g = psgp.tile([80, 80], F32, name="ps_g")
                for c in range(NCH):
                    xc = x_sb[b][:, c, :]
                    nc.tensor.matmul(
                        ps_g[:], xc, xc, start=(c == 0), stop=(c == NCH - 1)
                    )
                g_sb = evpool.tile([80, 80], F32, name=f"g{b}")
                nc.vector.tensor_copy(g_sb[:], ps_g[:])
                nc.gpsimd.dma_start(gram_d[b], g_sb[:])

            # ---- Y: y[j,f] = sum_e W[e,j] tk[e,f], fp8 DoubleRow.
            # psum group per (b, mat, fs-half): accumulate 4 chunk-pairs.
            for b in range(NB):
                y_sb = [
                    evpool.tile([40, E], BF16, name=f"y{b}_{m}") for m in range(2)
                ]
                for h in range(2):
                    for m in range(2):
                        ps = psyp.tile([48, 512], F32, name="ps_y")
                        tkt = tk_sb[(b, m, h)]
                        for j in range(NJ):
                            nc.tensor.matmul(
                                ps[:],
                                w8[b][:, 2 * j:2 * j + 2, :],
                                tkt[:, 2 * j:2 * j + 2, :],
                                start=(j == 0),
                                stop=(j == NJ - 1),
                                perf_mode=DR,
                            )
                        dst = y_sb[m][:, h * 512:(h + 1) * 512]
                        if m == 0:
                            nc.vector.tensor_copy(dst, ps[0:40, :])
                        else:
                            nc.scalar.copy(dst, ps[0:40, :])
                nc.gpsimd.dma_start(yr_d[b], y_sb[0][:])
                nc.gpsimd.dma_start(yi_d[b], y_sb[1][:])

    nc.compile()
    return nc


def _get_program():
    if "nc" not in _PROGRAM_CACHE:
        _PROGRAM_CACHE["nc"] = _build_program()
    return _PROGRAM_CACHE["nc"]


def _pack_inputs(nn, tkr, tki):
    """Host-side packing: per-core input dicts with device-friendly layouts."""
    # fp8, [B, E, E] -> [B, p, h, c, f512] with e = c*128+p, f = h*512+f512
    def pack_tk(x):
        x8 = x.astype(FP8_NP)
        x8 = x8.reshape(B, NCH, 128, 2, 512).transpose(0, 2, 3, 1, 4)
        return np.ascontiguousarray(x8)

    tkr8 = pack_tk(tkr)
    tki8 = pack_tk(tki)
    # [B, E, K] slices of nn
    Ur = nn[:, K:VLOC].reshape(B, E, K)
    Ui = nn[:, NOUT + K:NOUT + VLOC].reshape(B, E, K)
    Vr = nn[:, VLOC:NOUT].reshape(B, E, K)
    Vi = nn[:, NOUT + VLOC:2 * NOUT].reshape(B, E, K)
    xuv = np.concatenate([Ur, Ui, Vr, Vi], axis=2)        # [B, E, 80]
    # partition-major: e = c*128 + p  ->  [B, p, c, 80], fp16
    xuv = np.ascontiguousarray(
        xuv.reshape(B, NCH, 128, 80).transpose(0, 2, 1, 3).astype(np.float16)
    )
    return [
        {
            "xuv": xuv[i * NB:(i + 1) * NB],
            "tkr": tkr8[i * NB:(i + 1) * NB],
            "tki": tki8[i * NB:(i + 1) * NB],
        }
        for i in range(NCORES)
    ]


def _run_device(nn, tkr, tki, trace=False):
    nc = _get_program()
    in_maps = _pack_inputs(nn, tkr, tki)
    return run_bass_kernel_spmd(nc, in_maps, list(range(NCORES)), trace=trace)


def _den_host(tkr, tki):
    """den = ||tk||^2, exact float64 accumulation from the fp32 inputs."""
    acc = 0.0
    for x in (tkr, tki):
        rows = x.reshape(B, -1)
        for b in range(B):
            v = rows[b].astype(np.float64)
            acc += float(v @ v)
    return acc


def _finalize(nn, results, batch_size, den):
    """Assemble (loss, obj1, obj2) from per-core device partials (float64)."""
    nn = np.asarray(nn)
    d = (nn[:, :K] + 1j * nn[:, NOUT:NOUT + K]).astype(np.complex128)
    Vr = nn[:, VLOC:NOUT].reshape(B, E, K).astype(np.float64)
    Vi = nn[:, NOUT + VLOC:2 * NOUT].reshape(B, E, K).astype(np.float64)
    V = Vr + 1j * Vi

    gram = np.concatenate(
        [np.asarray(r["gram"]) for r in results], axis=0
    ).astype(np.float64)                                   # [B, 80, 80]
    yr = np.concatenate(
        [np.asarray(r["yr"]) for r in results], axis=0
    ).astype(np.float64)
    yi = np.concatenate(
        [np.asarray(r["yi"]) for r in results], axis=0
    ).astype(np.float64)

    SU = gram[:, 0:40, 0:40]
    SV = gram[:, 40:80, 40:80]
    Srr = SU[:, 0:20, 0:20]
    Sri = SU[:, 0:20, 20:40]
    Sii = SU[:, 20:40, 20:40]
    Trr = SV[:, 0:20, 0:20]
    Tri = SV[:, 0:20, 20:40]
    Tii = SV[:, 20:40, 20:40]
    SriT = np.transpose(Sri, (0, 2, 1))
    TriT = np.transpose(Tri, (0, 2, 1))
    G_U = (Srr - Sii) + 1j * (Sri + SriT)
    G_V = (Trr - Tii) + 1j * (Tri + TriT)
    H_U = (Srr + Sii) + 1j * (Sri - SriT)
    H_V = (Trr + Tii) + 1j * (Tri - TriT)

    mask = np.triu(np.ones((K, K), dtype=bool), k=1)
    bsz = float(batch_size)
    obj1 = float(np.sum(np.abs(G_U)[:, mask]) / bsz)
    obj2 = float(np.sum(np.abs(G_V)[:, mask]) / bsz)

    prednorm = float(
        np.real(
            np.einsum("bk,bl,bkl,bkl->", d, np.conj(d), np.conj(H_U), np.conj(H_V))
        )
    )

    # cross = Re<conj(tk), pred>; Wc[b,k,f] = sum_e conj(tk[e,f]) U[e,k]
    Wc = (yr[:, 0:20, :] + yi[:, 20:40, :]) + 1j * (yr[:, 20:40, :] - yi[:, 0:20, :])
    zeta = np.einsum("bfk,bkf->bk", V, Wc)
    cross = float(np.real(np.einsum("bk,bk->", d, zeta)))

    num = den - 2.0 * cross + prednorm
    loss = num / den + PENALTY * (obj1 + obj2)
    return (
        np.float32(loss),
        np.float32(obj1),
        np.float32(obj2),
    )


def kernel(nnOutput, kern_real, kern_imag, batch_Size):
    nn = np.ascontiguousarray(np.asarray(nnOutput, dtype=np.float32))
    tkr = np.asarray(kern_real, dtype=np.float32)
    tki = np.asarray(kern_imag, dtype=np.float32)
    den = _den_host(tkr, tki)
    res = _run_device(nn, tkr, tki).results
    return _finalize(nn, res, int(batch_Size), den)


# revision 3
# speedup vs baseline: 1.6517x; 1.0559x over previous
"""Trainium2 Bass kernel for nn_CustomLoss_74826920231413.

Loss structure (B=32, E=1024, K=20):
    c  = complex(nnOutput[:, :NOUT], nnOutput[:, NOUT:])
    d  = c[:, :K];  U = c[:, K:VLOC].reshape(B,E,K);  V = c[:, VLOC:].reshape(B,E,K)
    obj1/obj2 = sum_{j<k} |U^T U| / B (no conj), same for V
    pred = U @ diag(d) @ V^T;  tk = complex(kern_real, kern_imag)
    loss = ||tk - pred||^2 / ||tk||^2 + 0.01*(obj1+obj2)

Device strategy (data-parallel over B, 4 batch rows per core, 8 cores):
    ||tk - pred||^2 = ||tk||^2 - 2*Re<conj(tk),pred> + ||pred||^2.  The
    device streams tk once, producing small outputs:
      * gram[b] = X^T X with X = [Ur|Ui|Vr|Vi]   -> objs, ||pred||^2
      * yr[b]   = W^T tkr with W = [Ur|Ui]       -> cross term
      * yi[b]   = W^T tki
    den = ||tk||^2 is an exact scalar the host computes in float64 from
    the fp32 originals (cheap BLAS dot); the loss is a ratio of O(1e9)
    sums, so the cross/pred partials tolerate aggressive quantization
    (validated against the fp64 reference at ~1e-6..1e-4 relative):
      * tk and xuv ride as fp8 e4m3 -> half the fp16 HBM traffic.
      * Y matmuls run in fp8 DoubleRow perf mode (2 k-chunks per
        instruction, 2 fp8/cell/cycle) so PE stays under the DMA
        envelope; the Gram runs on the same fp8 data (PE upconverts to
        fp22, fp32 accumulate).
      * y evacuates as fp8 (scaled 1/32 against the 240 saturation
        point), gram as fp16.
    tkr streams on the sync HWDGE ring, tki+xuv on the act HWDGE ring,
    so both hardware descriptor rings feed HBM concurrently; outputs
    ride the same rings behind the inputs (FIFO), avoiding the slow
    SWDGE tail.  Everything is SBUF-resident (~80KB/partition), so all
    input DMAs are issued up-front and the rings run back-to-back at
    the HBM-per-core limit with the PE trailing just behind.
"""

import sys

for _p in ("/opt/trn_rl_repo", "/root/.axon_site/_ro/trn_rl_repo"):
    if _p not in sys.path:
        sys.path.append(_p)

import ml_dtypes
import numpy as np

import concourse.bacc as bacc
import concourse.mybir as mybir
import concourse.tile as tile
from concourse.bass_utils import run_bass_kernel_spmd

# Problem constants (hardcoded per harness contract)
E = 1024
K = 20
NOUT = K * (2 * E + 1)          # 40980
VLOC = K + K * E                # 20500
PENALTY = 0.01
B = 32
NCORES = 8
NB = B // NCORES                # batch rows per core
NCH = E // 128                  # 8 e-chunks of 128 partitions
NJ = NCH // 2                   # 4 chunk-pairs (DoubleRow contracts 2)
YSCALE = 1.0 / 32.0             # fp8 evac scale for y outputs
F32 = mybir.dt.float32
F16 = mybir.dt.float16
FP8 = mybir.dt.float8e4
FP8_NP = ml_dtypes.float8_e4m3fn
DR = mybir.MatmulPerfMode.DoubleRow

_PROGRAM_CACHE = {}


def _build_program():
    """Per-core SPMD Bass program. Same program on all 8 cores; each core
    receives its own 4-row slice of the inputs (host-packed layouts)."""
    nc = bacc.Bacc("TRN2", target_bir_lowering=False, debug=False)

    # host-packed [Ur|Ui|Vr|Vi] fp8, partition-major: [b, p, c, 80]
    xuv_d = nc.dram_tensor("xuv", [NB, 128, NCH, 80], FP8, kind="ExternalInput").ap()
    # host-packed fp8 kernels: [b, p, h, c, f512], e = c*128+p, f = h*512+f512.
    # 4KB contiguous per (b,p,h) line -> efficient DMA descriptors.
    tkr_d = nc.dram_tensor(
        "tkr", [NB, 128, 2, NCH, 512], FP8, kind="ExternalInput"
    ).ap()
    tki_d = nc.dram_tensor(
        "tki", [NB, 128, 2, NCH, 512], FP8, kind="ExternalInput"
    ).ap()

    gram_d = nc.dram_tensor("gram", [NB, 80, 80], F16, kind="ExternalOutput").ap()
    yr_d = nc.dram_tensor("yr", [NB, 40, E], FP8, kind="ExternalOutput").ap()
    yi_d = nc.dram_tensor("yi", [NB, 40, E], FP8, kind="ExternalOutput").ap()

    with tile.TileContext(nc) as tc:
        with (
            tc.tile_pool(name="x", bufs=1) as xpool,
            tc.tile_pool(name="tk", bufs=1) as tkpool,
            tc.tile_pool(name="ev", bufs=1) as evpool,
            tc.tile_pool(name="psg", bufs=2, space="PSUM") as psgp,
            tc.tile_pool(name="psy", bufs=6, space="PSUM") as psyp,
        ):
            # ---- all input DMAs issued up-front, in consumption order.
            # act ring: xuv then tki halves; sync ring: tkr halves.
            x_sb = []
            for b in range(NB):
                t = xpool.tile([128, NCH, 80], FP8, name=f"x{b}")
                nc.scalar.dma_start(t[:], xuv_d[b])
                x_sb.append(t)
            tk_sb = {}
            for b in range(NB):
                for h in range(2):
                    t = tkpool.tile([128, NCH, 512], FP8, name=f"tkr{b}_{h}")
                    nc.sync.dma_start(t[:], tkr_d[b, :, h])
                    tk_sb[(b, 0, h)] = t
                    t = tkpool.tile([128, NCH, 512], FP8, name=f"tki{b}_{h}")
                    nc.scalar.dma_start(t[:], tki_d[b, :, h])
                    tk_sb[(b, 1, h)] = t

            # ---- Grams: [80,80] = X^T X per b (fp8 in, fp32 accum).
            for b in range(NB):
                ps_g = psgp.tile([80, 80], F32, name="ps_g")
                for c in range(NCH):
                    xc = x_sb[b][:, c, :]
                    nc.tensor.matmul(
                        ps_g[:], xc, xc, start=(c == 0), stop=(c == NCH - 1)
                    )
                g_sb = evpool.tile([80, 80], F16, name=f"g{b}")
                nc.vector.tensor_copy(g_sb[:], ps_g[:])
                nc.scalar.dma_start(gram_d[b], g_sb[:])

            # ---- Y: y[j,f] = sum_e W[e,j] tk[e,f], fp8 DoubleRow with
            # W = xuv cols 0:40 = [Ur|Ui] sliced in place.
            for b in range(NB):
                y_sb = [
                    evpool.tile([40, E], FP8, name=f"y{b}_{m}") for m in range(2)
                ]
                for h in range(2):
                    for m in range(2):
                        ps = psyp.tile([40, 512], F32, name="ps_y")
                        tkt = tk_sb[(b, m, h)]
                        for j in range(NJ):
                            nc.tensor.matmul(
                                ps[:],
                                x_sb[b][:, 2 * j:2 * j + 2, 0:40],
                                tkt[:, 2 * j:2 * j + 2, :],
                                start=(j == 0),
                                stop=(j == NJ - 1),
                                perf_mode=DR,
                            )
                        dst = y_sb[m][:, h * 512:(h + 1) * 512]
                        if m == 0:
                            nc.vector.tensor_scalar_mul(dst, ps[:], YSCALE)
                        else:
                            nc.scalar.mul(dst, ps[:], YSCALE)
                nc.sync.dma_start(yr_d[b], y_sb[0][:])
                nc.sync.dma_start(yi_d[b], y_sb[1][:])

    nc.compile()
    return nc


def _get_program():
    if "nc" not in _PROGRAM_CACHE:
        _PROGRAM_CACHE["nc"] = _build_program()
    return _PROGRAM_CACHE["nc"]


def _pack_inputs(nn, tkr, tki):
    """Host-side packing: per-core input dicts with device-friendly layouts."""
    # fp8, [B, E, E] -> [B, p, h, c, f512] with e = c*128+p, f = h*512+f512
    def pack_tk(x):
        x8 = x.astype(FP8_NP)
        x8 = x8.reshape(B, NCH, 128, 2, 512).transpose(0, 2, 3, 1, 4)
        return np.ascontiguousarray(x8)

    tkr8 = pack_tk(tkr)
    tki8 = pack_tk(tki)
    # [B, E, K] slices of nn
    Ur = nn[:, K:VLOC].reshape(B, E, K)
    Ui = nn[:, NOUT + K:NOUT + VLOC].reshape(B, E, K)
    Vr = nn[:, VLOC:NOUT].reshape(B, E, K)
    Vi = nn[:, NOUT + VLOC:2 * NOUT].reshape(B, E, K)
    xuv = np.concatenate([Ur, Ui, Vr, Vi], axis=2)        # [B, E, 80]
    # partition-major: e = c*128 + p  ->  [B, p, c, 80], fp8
    xuv = np.ascontiguousarray(
        xuv.reshape(B, NCH, 128, 80).transpose(0, 2, 1, 3).astype(FP8_NP)
    )
    return [
        {
            "xuv": xuv[i * NB:(i + 1) * NB],
            "tkr": tkr8[i * NB:(i + 1) * NB],
            "tki": tki8[i * NB:(i + 1) * NB],
        }
        for i in range(NCORES)
    ]


def _run_device(nn, tkr, tki, trace=False):
    nc = _get_program()
    in_maps = _pack_inputs(nn, tkr, tki)
    return run_bass_kernel_spmd(nc, in_maps, list(range(NCORES)), trace=trace)


def _den_host(tkr, tki):
    """den = ||tk||^2, exact float64 accumulation from the fp32 inputs."""
    acc = 0.0
    for x in (tkr, tki):
        rows = x.reshape(B, -1)
        for b in range(B):
            v = rows[b].astype(np.float64)
            acc += float(v @ v)
    return acc


def _finalize(nn, results, batch_size, den):
    """Assemble (loss, obj1, obj2) from per-core device partials (float64)."""
    nn = np.asarray(nn)
    d = (nn[:, :K] + 1j * nn[:, NOUT:NOUT + K]).astype(np.complex128)
    Vr = nn[:, VLOC:NOUT].reshape(B, E, K).astype(np.float64)
    Vi = nn[:, NOUT + VLOC:2 * NOUT].reshape(B, E, K).astype(np.float64)
    V = Vr + 1j * Vi

    gram = np.concatenate(
        [np.asarray(r["gram"]) for r in results], axis=0
    ).astype(np.float64)                                   # [B, 80, 80]
    yr = np.concatenate(
        [np.asarray(r["yr"]) for r in results], axis=0
    ).astype(np.float64) / YSCALE
    yi = np.concatenate(
        [np.asarray(r["yi"]) for r in results], axis=0
    ).astype(np.float64) / YSCALE

    SU = gram[:, 0:40, 0:40]
    SV = gram[:, 40:80, 40:80]
    Srr = SU[:, 0:20, 0:20]
    Sri = SU[:, 0:20, 20:40]
    Sii = SU[:, 20:40, 20:40]
    Trr = SV[:, 0:20, 0:20]
    Tri = SV[:, 0:20, 20:40]
    Tii = SV[:, 20:40, 20:40]
    SriT = np.transpose(Sri, (0, 2, 1))
    TriT = np.transpose(Tri, (0, 2, 1))
    G_U = (Srr - Sii) + 1j * (Sri + SriT)
    G_V = (Trr - Tii) + 1j * (Tri + TriT)
    H_U = (Srr + Sii) + 1j * (Sri - SriT)
    H_V = (Trr + Tii) + 1j * (Tri - TriT)

    mask = np.triu(np.ones((K, K), dtype=bool), k=1)
    bsz = float(batch_size)
    obj1 = float(np.sum(np.abs(G_U)[:, mask]) / bsz)
    obj2 = float(np.sum(np.abs(G_V)[:, mask]) / bsz)

    prednorm = float(
        np.real(
            np.einsum("bk,bl,bkl,bkl->", d, np.conj(d), np.conj(H_U), np.conj(H_V))
        )
    )

    # cross = Re<conj(tk), pred>; Wc[b,k,f] = sum_e conj(tk[e,f]) U[e,k]
    Wc = (yr[:, 0:20, :] + yi[:, 20:40, :]) + 1j * (yr[:, 20:40, :] - yi[:, 0:20, :])
    zeta = np.einsum("bfk,bkf->bk", V, Wc)
    cross = float(np.real(np.einsum("bk,bk->", d, zeta)))

    num = den - 2.0 * cross + prednorm
    loss = num / den + PENALTY * (obj1 + obj2)
    return (
        np.float32(loss),
        np.float32(obj1),
        np.float32(obj2),
    )


def kernel(nnOutput, kern_real, kern_imag, batch_Size):
    nn = np.ascontiguousarray(np.asarray(nnOutput, dtype=np.float32))
    tkr = np.asarray(kern_real, dtype=np.float32)
    tki = np.asarray(kern_imag, dtype=np.float32)
    den = _den_host(tkr, tki)
    res = _run_device(nn, tkr, tki).results
    return _finalize(nn, res, int(batch_Size), den)
